# revision 8
# baseline (speedup 1.0000x reference)
"""Contrastive loss kernel for Trainium2 (8 NeuronCores, SPMD via bass).

Strategy:
  * Host sorts the batch by label. The loss is invariant under a joint
    row/column permutation, so no unpermute is needed. After sorting, the
    "same-label" mask becomes one contiguous column range per label.
  * Launch A (data-parallel over rows): host pre-transposes the embedding
    shard to embT (bf16); each core computes eT = W @ embT + b, row norms
    via a ones-matmul over squares, rni = exp(-0.5*ln(|e|^2)) on the ACT
    engine, enT = eT * rni (bf16 out), and S' = eT @ lnT (unnormalized;
    host multiplies by rni in the float64 finalize).
  * Host gathers enT, builds per-core label-aligned 128-row blocks with the
    same-label column range zero-padded to a uniform width W_s. Padding is
    corrected exactly on device: a zero column contributes C=0, exp(C)=1.
  * Launch B: each core processes B block-slots; for each it computes the
    [<=128, 8192] cosine block with fp8 DoubleRow matmuls (K=256 in one
    instruction), exp row-sums via ACT accum_out, and the same-range terms
    using the first-order expansion
        sum_j ln(negsum + exp(Cs_j)) ~= W_s*ln(negsum) + ss/negsum
    (exact to ~1e-7 since exp(C)/negsum <= 3e-4), so no per-element ln
    pass is needed. csr = sum_j Cs_j comes from a tiny matmul against
    host-precomputed same-range column sums.
  * Host: inter = sum(terms)/bs^2; l1/l2 finalized from S in float64
    (O(bs*L) work); loss = 0.5*inter + 0.5*(l1+l2).

Both launches use only Exp/Ln/Identity/Square activations; a compile-time
activation-table override makes bass pick the combined
natural_log_exp_and_others set so each launch issues exactly one
ACT_TABLE_LOAD (the greedy default alternates sets, costing ~1.4us per
switch).
"""

import contextlib
import math
import os

import ml_dtypes
import numpy as np

os.environ.setdefault("NEURON_RT_VIRTUAL_CORE_SIZE", "1")

import concourse.bass as bass
import concourse.mybir as mybir
from concourse import bacc
import concourse.tile as tile
from concourse.bass_utils import run_bass_kernel_spmd

BS = 8192
D_IN = 1024
D_EMB = 256
L = 10
NC = 8
P = 128
RPC = BS // NC          # rows per core in launch A
RT = RPC // P           # row tiles per core (8)
KT = D_IN // P          # k tiles (8)
KM = D_EMB // P         # emb-dim partition chunks (2)
GA = 2                  # launch A row groups (512 rows each)
RG = RPC // GA

F32 = mybir.dt.float32
BF16 = mybir.dt.bfloat16
BF16_NP = ml_dtypes.bfloat16
F8 = mybir.dt.float8e4
F8_NP = ml_dtypes.float8_e4m3
F8_SCALE = 16.0
AX = mybir.AxisListType.X
AF = mybir.ActivationFunctionType
DR = mybir.MatmulPerfMode.DoubleRow

# Results of the last kernel() call (for test.py introspection/timing).
LAST = {}


@contextlib.contextmanager
def _combined_act_tables():
    """Make the act-table pass resolve Exp and Ln to the combined
    natural_log_exp_and_others set (index preserved), so kernels using
    both emit a single ACT_TABLE_LOAD instead of alternating sets."""
    orig = bacc.get_activation_tables

    def patched(arch):
        t = orig(arch)
        E = mybir.ActivationFunctionType.Exp
        Ln = mybir.ActivationFunctionType.Ln
        return {
            k: (v if k == "natural_log_exp_and_others" else v - {E, Ln})
            for k, v in t.items()
        }

    bacc.get_activation_tables = patched
    try:
        yield
    finally:
        bacc.get_activation_tables = orig


# --------------------------------------------------------------------------
# Launch A: per-core transform  embT[1024,1024]bf16 -> enT[256,1024] bf16,
# S'[1024,10] f32 (unnormalized), rni[1024] f32
# --------------------------------------------------------------------------
def build_launch_a():
    nc = bacc.Bacc("TRN2", target_bir_lowering=False, debug=False, num_devices=NC)
    embt_d = nc.dram_tensor("embt", [P, GA * KT * RG], BF16, kind="ExternalInput")
    wt_d = nc.dram_tensor("wt", [P, KT * D_EMB], BF16, kind="ExternalInput")
    bias_d = nc.dram_tensor("bias", [P, KM], F32, kind="ExternalInput")
    lnt_d = nc.dram_tensor("lnt", [P, KM * L], F32, kind="ExternalInput")
    ent_d = nc.dram_tensor("ent_out", [P, KM * RPC], BF16, kind="ExternalOutput")
    s_d = nc.dram_tensor("s_out", [P, RT * L], F32, kind="ExternalOutput")
    rni_d = nc.dram_tensor("rni_out", [1, RPC], F32, kind="ExternalOutput")

    NCH = RPC // 512        # 512-wide column chunks of eT (2)
    with tile.TileContext(nc) as tc:
        with (
            tc.tile_pool(name="const", bufs=1) as cpool,
            tc.tile_pool(name="big", bufs=1) as big_pool,
            tc.tile_pool(name="sq", bufs=4) as sq_pool,
            tc.tile_pool(name="sml", bufs=2) as sml_pool,
            tc.tile_pool(name="ps", bufs=1, space="PSUM") as ps_pool,
        ):
            ones_c = cpool.tile([P, 1], BF16)
            nc.vector.memset(ones_c[:], 1.0)
            ones_r = cpool.tile([1, P], F32)
            nc.vector.memset(ones_r[:], 1.0)

            wt_sb = cpool.tile([P, KT * D_EMB], BF16)
            nc.gpsimd.dma_start(wt_sb[:], wt_d.ap())
            b_sb = cpool.tile([P, KM], F32)
            nc.gpsimd.dma_start(b_sb[:], bias_d.ap())
            lnt_sb = cpool.tile([P, KM * L], F32)
            nc.gpsimd.dma_start(lnt_sb[:], lnt_d.ap())

            embt_sb = big_pool.tile([P, GA, KT, RG], BF16)
            for g in range(GA):
                nc.gpsimd.dma_start(
                    embt_sb[:, g, :, :],
                    embt_d.ap()[:, g * KT * RG:(g + 1) * KT * RG],
                )

            # --- eT = W @ embT + b; norms^2 via ones-matmul over squares ---
            eT = [big_pool.tile([P, RPC], F32, name=f"eT{m}") for m in range(KM)]
            psn = [ps_pool.tile([1, 512], F32, name=f"psn{g}", tag=f"psn{g}")
                   for g in range(GA)]
            for g in range(GA):
                for m in range(KM):
                    pe = ps_pool.tile([P, 512], F32, tag="peA", bufs=2)
                    for kc in range(KT):
                        nc.tensor.matmul(
                            pe[:],
                            wt_sb[:, kc * D_EMB + m * P: kc * D_EMB + (m + 1) * P],
                            embt_sb[:, g, kc, :],
                            start=(kc == 0),
                            stop=(kc == KT - 1),
                        )
                    nc.scalar.activation(
                        eT[m][:, g * 512:(g + 1) * 512], pe[:],
                        AF.Identity, bias=b_sb[:, m:m + 1],
                    )
                    esq = sq_pool.tile([P, 512], BF16)
                    nc.vector.tensor_mul(
                        esq[:],
                        eT[m][:, g * 512:(g + 1) * 512],
                        eT[m][:, g * 512:(g + 1) * 512],
                    )
                    nc.tensor.matmul(
                        psn[g][:],
                        ones_c[:],
                        esq[:],
                        start=(m == 0),
                        stop=(m == KM - 1),
                    )

            # --- rni = exp(-0.5 * ln(|e|^2)) on ACT (all lanes idle; [1,*]) ---
            lnsq = sml_pool.tile([1, RPC], F32)
            for g in range(GA):
                nc.scalar.activation(
                    lnsq[:, g * 512:(g + 1) * 512], psn[g][:], AF.Ln,
                )
            rni = sml_pool.tile([1, RPC], F32)
            nc.scalar.activation(rni[:], lnsq[:], AF.Exp, scale=-0.5)
            nc.gpsimd.dma_start(rni_d.ap(), rni[:])

            # --- S' = eT.T @ lnT (unnormalized; fp32, tiny free dim) ---
            s_sb = sml_pool.tile([P, RT * L], F32)
            for r in range(RT):
                pss = ps_pool.tile([P, L], F32, tag="pss", bufs=2)
                for m in range(KM):
                    nc.tensor.matmul(
                        pss[:],
                        eT[m][:, r * P:(r + 1) * P],
                        lnt_sb[:, m * L:(m + 1) * L],
                        start=(m == 0),
                        stop=(m == KM - 1),
                    )
                nc.vector.tensor_copy(s_sb[:, r * L:(r + 1) * L], pss[:])
            nc.gpsimd.dma_start(s_d.ap(), s_sb[:])

            # --- enT = eT * rni (broadcast via K=1 matmul), bf16 out ---
            enT_sb = big_pool.tile([P, KM * RPC], BF16)
            for g in range(GA):
                psb = ps_pool.tile([P, 512], F32, tag="psb", bufs=2)
                nc.tensor.matmul(
                    psb[:], ones_r[:], rni[:, g * 512:(g + 1) * 512],
                    start=True, stop=True,
                )
                for m in range(KM):
                    nc.vector.tensor_mul(
                        enT_sb[:, m * RPC + g * 512: m * RPC + (g + 1) * 512],
                        eT[m][:, g * 512:(g + 1) * 512],
                        psb[:],
                    )
            nc.gpsimd.dma_start(ent_d.ap(), enT_sb[:])

    with _combined_act_tables():
        nc.compile()
    return nc


# --------------------------------------------------------------------------
# Launch B: per-core B block-slots of the inter-sample loss
# --------------------------------------------------------------------------
def build_launch_b(B, W_s):
    WH = W_s // 512  # samerange column halves
    nc = bacc.Bacc("TRN2", target_bir_lowering=False, debug=False, num_devices=NC)
    ent_d = nc.dram_tensor("ent", [P, KM * BS], F8, kind="ExternalInput")
    lhst_d = nc.dram_tensor("lhst", [P, KM * B * P], F8, kind="ExternalInput")
    rs_d = nc.dram_tensor("rsame", [P, KM * B * W_s], F8, kind="ExternalInput")
    wsum_d = nc.dram_tensor("wsum", [P, KM * B], F8, kind="ExternalInput")
    meta_d = nc.dram_tensor("meta", [P, 3 * B], F32, kind="ExternalInput")
    terms_d = nc.dram_tensor("terms", [P, B], F32, kind="ExternalOutput")

    CW = 2048                  # psum/ACT chunk width (4 banks)
    NG = BS // CW              # main chunk groups (4)
    INV_FS2 = 1.0 / (F8_SCALE * F8_SCALE)
    with tile.TileContext(nc) as tc:
        with (
            tc.tile_pool(name="inp", bufs=1) as inp_pool,
            tc.tile_pool(name="escr", bufs=2) as escr_pool,
            tc.tile_pool(name="rsp", bufs=2) as rsp_pool,
            tc.tile_pool(name="sml", bufs=2) as sml_pool,
            tc.tile_pool(name="psm", bufs=2, space="PSUM") as psm_pool,
        ):
            ent_sb = inp_pool.tile([P, NG, KM, CW], F8)
            lhst_sb = inp_pool.tile([P, KM, B, P], F8)
            # rsame is b-outer [P, (b, m, w)] so per-b DMA chunks are contiguous
            rs_sb = inp_pool.tile([P, B, KM, W_s], F8)
            wsum_sb = inp_pool.tile([P, KM, B], F8)
            meta_sb = inp_pool.tile([P, 3 * B], F32)
            nc.gpsimd.dma_start(meta_sb[:], meta_d.ap())
            nc.gpsimd.dma_start(wsum_sb[:, :, :], wsum_d.ap())
            nc.gpsimd.dma_start(lhst_sb[:, :, :, :], lhst_d.ap())
            # rsame chunked by b-thirds so block 0 can start early
            third = max(1, B // 3)
            bounds = [0, third, 2 * third, B]
            for i in range(3):
                lo, hi = bounds[i], bounds[i + 1]
                if lo < hi:
                    nc.gpsimd.dma_start(
                        rs_sb[:, lo:hi, :, :],
                        rs_d.ap()[:, lo * KM * W_s: hi * KM * W_s],
                    )
            # ent grouped by 2048-column blocks: [(g, m) -> CW cols]; chunked
            # DMAs so main-row group g can start once its chunk lands
            for g in range(NG):
                nc.gpsimd.dma_start(
                    ent_sb[:, g, :, :],
                    ent_d.ap()[:, g * KM * CW:(g + 1) * KM * CW],
                )
            coef_sb = meta_sb[:, 0:B]
            pad_sb = meta_sb[:, B:2 * B]
            mask_sb = meta_sb[:, 2 * B:3 * B]
            terms_sb = inp_pool.tile([P, B], F32)
            e_const = inp_pool.tile([P, 1], F32)
            nc.vector.memset(e_const[:], float(np.e))

            for b in range(B):
                lhs = lhst_sb[:, :, b, :]        # [128, 2, 128] fp8

                # same-label column range C block; csr matmul shares the
                # unused tail of the same PSUM tile (W_s < CW)
                ps_s = psm_pool.tile([P, CW], F32, tag="psbig", bufs=2)
                ps_w = ps_s[:, W_s:W_s + 1]
                for m in range(KM):
                    nc.tensor.matmul(
                        ps_w,
                        lhst_sb[:, m, b, :],
                        wsum_sb[:, m, b:b + 1],
                        start=(m == 0), stop=(m == KM - 1),
                    )
                for h in range(WH):
                    nc.tensor.matmul(
                        ps_s[:, h * 512:(h + 1) * 512],
                        lhs,
                        rs_sb[:, b, :, h * 512:(h + 1) * 512],
                        start=True, stop=True, perf_mode=DR,
                    )
                es = escr_pool.tile([P, W_s], BF16, tag="es", bufs=2)
                ss = sml_pool.tile([P, 1], F32)
                nc.scalar.activation(
                    es[:], ps_s[:, :W_s], AF.Exp, accum_out=ss[:],
                    scale=INV_FS2,
                )

                # full row: C chunks + exp row-sums
                rsp = rsp_pool.tile([P, NG], F32)
                for g in range(NG):
                    ps_c = psm_pool.tile([P, CW], F32, tag="psbig", bufs=2)
                    for nn in range(CW // 512):
                        nc.tensor.matmul(
                            ps_c[:, nn * 512:(nn + 1) * 512],
                            lhs,
                            ent_sb[:, g, :, nn * 512:(nn + 1) * 512],
                            start=True, stop=True, perf_mode=DR,
                        )
                    e_scr = escr_pool.tile([P, CW], BF16)
                    nc.scalar.activation(
                        e_scr[:], ps_c[:], AF.Exp, accum_out=rsp[:, g:g + 1],
                        scale=INV_FS2,
                    )

                # per-row assembly
                rs_all = sml_pool.tile([P, 1], F32)
                nc.vector.reduce_sum(rs_all[:], rsp[:], axis=AX)
                negsum = sml_pool.tile([P, 1], F32)
                nc.vector.tensor_sub(negsum[:], rs_all[:], ss[:])
                nc.vector.tensor_add(negsum[:], negsum[:], pad_sb[:, b:b + 1])

                # ln(negsum+1), ln(negsum+e), ln(negsum) in one ACT call
                ladd = sml_pool.tile([P, 3], F32)
                nc.vector.tensor_scalar_add(ladd[:, 0:1], negsum[:], 1.0)
                nc.vector.tensor_add(ladd[:, 1:2], negsum[:], e_const[:])
                nc.vector.tensor_copy(ladd[:, 2:3], negsum[:])
                lout = sml_pool.tile([P, 3], F32)
                nc.scalar.activation(lout[:], ladd[:], AF.Ln)
                lt = lout[:, 0:1]
                le = lout[:, 1:2]
                lnn = lout[:, 2:3]

                # first-order: sum_j ln(negsum+exp(Cs_j)) = W_s*lnn + ss/negsum
                rec = sml_pool.tile([P, 1], F32)
                nc.vector.reciprocal(rec[:], negsum[:])
                lnsum = sml_pool.tile([P, 1], F32)
                nc.vector.tensor_mul(lnsum[:], ss[:], rec[:])
                wlnn = sml_pool.tile([P, 1], F32)
                nc.vector.tensor_scalar_mul(wlnn[:], lnn, float(W_s))
                nc.vector.tensor_add(lnsum[:], lnsum[:], wlnn[:])

                # term = coef*lt + (lnsum - csr) - le + 1, masked
                csr = sml_pool.tile([P, 1], F32)
                nc.vector.tensor_scalar_mul(csr[:], ps_w, INV_FS2)
                t1 = sml_pool.tile([P, 1], F32)
                nc.vector.tensor_mul(t1[:], coef_sb[:, b:b + 1], lt)
                nc.vector.tensor_add(t1[:], t1[:], lnsum[:])
                nc.vector.tensor_sub(t1[:], t1[:], csr[:])
                nc.vector.tensor_sub(t1[:], t1[:], le)
                nc.vector.tensor_scalar_add(t1[:], t1[:], 1.0)
                nc.vector.tensor_mul(terms_sb[:, b:b + 1], t1[:], mask_sb[:, b:b + 1])

            nc.gpsimd.dma_start(terms_d.ap(), terms_sb[:])

    with _combined_act_tables():
        nc.compile()
    return nc


# --------------------------------------------------------------------------
# Host orchestration
# --------------------------------------------------------------------------
def _plan_blocks(labels_s):
    counts = np.bincount(labels_s.astype(np.int64), minlength=L)
    starts = np.concatenate([[0], np.cumsum(counts)[:-1]])
    blocks = []
    for lab in range(L):
        s, c = int(starts[lab]), int(counts[lab])
        for off in range(0, c, P):
            blocks.append((s + off, min(P, c - off), lab))
    B = math.ceil(len(blocks) / NC)
    W_s = max(1024, math.ceil((int(counts.max()) if len(blocks) else 1) / 512) * 512)
    return blocks, counts, starts, B, W_s


def _pm(a):
    """[G, P, N] -> partition-major [P, G*N]."""
    g, p, n = a.shape
    return np.ascontiguousarray(a.transpose(1, 0, 2).reshape(p, g * n))


def _prep_launch_a_inputs(emb_s, W, b, label_emb):
    wt = _pm(np.ascontiguousarray(W.T).reshape(KT, P, D_EMB)).astype(BF16_NP)
    bias = np.ascontiguousarray(b.reshape(KM, P).T).astype(np.float32)
    ln = label_emb / np.maximum(
        np.sqrt((label_emb.astype(np.float64) ** 2).sum(-1, keepdims=True)), 1e-8
    )
    lnt = _pm(np.ascontiguousarray(ln.T).reshape(KM, P, L)).astype(np.float32)
    in_maps = []
    for c in range(NC):
        sh = emb_s[c * RPC:(c + 1) * RPC].astype(BF16_NP)  # [1024, 1024] bf16
        # embT layout [P, (g, kc, r)]: embt[p, g, kc, r] = sh[g*RG + r, kc*128+p]
        et = np.ascontiguousarray(
            sh.reshape(GA, RG, KT, P).transpose(3, 0, 2, 1).reshape(P, GA * KT * RG)
        )
        in_maps.append({"embt": et, "wt": wt, "bias": bias, "lnt": lnt})
    return in_maps


def _prep_launch_b_inputs(enT_full, blocks, counts, starts, B, W_s):
    CW = 2048
    NG = BS // CW
    entf = enT_full.astype(np.float32)
    ent8 = (entf * F8_SCALE).astype(F8_NP)
    ent3 = ent8.reshape(KM, P, BS)
    # [P, (g, m, cw)] grouping to match the chunked DMAs
    ent = np.ascontiguousarray(
        ent8.reshape(KM, P, NG, CW).transpose(1, 2, 0, 3).reshape(P, NG * KM * CW)
    )
    in_maps = []
    for c in range(NC):
        blks = blocks[c * B:(c + 1) * B]
        lhst = np.zeros((KM, P, B * P), F8_NP)
        rsame = np.zeros((KM, P, B, W_s), F8_NP)
        wsum = np.zeros((KM, P, B), np.float32)
        meta = np.zeros((P, 3 * B), np.float32)
        for i, (rs, w, lab) in enumerate(blks):
            lhst[:, :, i * P:i * P + w] = ent3[:, :, rs:rs + w]
            s, cnt = int(starts[lab]), int(counts[lab])
            rsame[:, :, i, :cnt] = ent3[:, :, s:s + cnt]
            # same-range column sums (x F8_SCALE) for the csr matmul
            wsum[:, :, i] = (
                entf.reshape(KM, P, BS)[:, :, s:s + cnt].sum(axis=2) * F8_SCALE
            )
            meta[:w, i] = BS - W_s            # coef
            meta[:, B + i] = W_s - cnt        # pad
            meta[:w, 2 * B + i] = 1.0         # mask
        rsame_pm = np.ascontiguousarray(
            rsame.transpose(1, 2, 0, 3).reshape(P, B * KM * W_s)
        )
        in_maps.append({
            "ent": ent, "lhst": _pm(lhst), "rsame": rsame_pm,
            "wsum": _pm(wsum.astype(F8_NP)), "meta": meta,
        })
    return in_maps


def _finalize_l1_l2(S_sorted, labels_s):
    S = S_sorted.astype(np.float64)
    idx = np.arange(BS)
    lab = labels_s.astype(np.int64)
    Pv = S[idx, lab]
    E2 = np.exp(S)
    eP = np.exp(Pv)
    neg1 = E2.sum(axis=1) - eP
    col_tot = E2.sum(axis=0)
    own_col = np.bincount(lab, weights=eP, minlength=L)
    neg2 = (col_tot - own_col)[lab]
    l1 = np.mean(-Pv + np.log(neg1 + eP))
    l2 = np.mean(-Pv + np.log(neg2 + eP))
    return l1, l2


def kernel(embedding, labels, W, b, label_emb):
    embedding = np.asarray(embedding, np.float32)
    labels_np = np.asarray(labels)
    W = np.asarray(W, np.float32)
    b = np.asarray(b, np.float32)
    label_emb = np.asarray(label_emb, np.float32)

    perm = np.argsort(labels_np, kind="stable")
    labels_s = labels_np[perm]
    emb_s = embedding[perm]
    blocks, counts, starts, B, W_s = _plan_blocks(labels_s)

    # ---- launch A ----
    nc_a = build_launch_a()
    in_maps_a = _prep_launch_a_inputs(emb_s, W, b, label_emb)
    res_a = run_bass_kernel_spmd(nc_a, in_maps_a, core_ids=list(range(NC)))
    LAST["a"] = res_a

    enT_full = np.empty((D_EMB, BS), BF16_NP)
    S_sorted = np.empty((BS, L), np.float64)
    for c in range(NC):
        out = res_a.results[c]
        ent_c = np.asarray(out["ent_out"])  # [P, KM*RPC]
        for m in range(KM):
            enT_full[m * P:(m + 1) * P, c * RPC:(c + 1) * RPC] = \
                ent_c[:, m * RPC:(m + 1) * RPC]
        s_c = np.asarray(out["s_out"]).reshape(P, RT, L)
        rni_c = np.asarray(out["rni_out"]).reshape(RPC)
        S_sorted[c * RPC:(c + 1) * RPC] = (
            s_c.transpose(1, 0, 2).reshape(RPC, L).astype(np.float64)
            * rni_c[:, None].astype(np.float64)
        )

    # ---- launch B ----
    nc_b = build_launch_b(B, W_s)
    in_maps_b = _prep_launch_b_inputs(enT_full, blocks, counts, starts, B, W_s)
    res_b = run_bass_kernel_spmd(nc_b, in_maps_b, core_ids=list(range(NC)))
    LAST["b"] = res_b

    total = 0.0
    for c in range(NC):
        total += np.asarray(res_b.results[c]["terms"], np.float64).sum()
    inter = total / (BS * BS)

    l1, l2 = _finalize_l1_l2(S_sorted, labels_s)
    return np.float32(0.5 * inter + 0.5 * (l1 + l2))


# revision 15
# speedup vs baseline: 1.0173x; 1.0173x over previous
"""Contrastive loss kernel for Trainium2 (8 NeuronCores, SPMD via bass).

Strategy:
  * Host sorts the batch by label. The loss is invariant under a joint
    row/column permutation, so no unpermute is needed. After sorting, the
    "same-label" mask becomes one contiguous column range per label.
  * Launch A (data-parallel over rows): host pre-transposes the embedding
    shard to embT (bf16); each core computes eT = W @ embT + b, row norms
    via a ones-matmul over squares, rni = exp(-0.5*ln(|e|^2)) on the ACT
    engine, enT = eT * rni (bf16 out), and S' = eT @ lnT (unnormalized;
    host multiplies by rni in the float64 finalize).
  * Host gathers enT, builds per-core label-aligned 128-row blocks with the
    same-label column range zero-padded to a uniform width W_s. Padding is
    corrected exactly on device: a zero column contributes C=0, exp(C)=1.
  * Launch B: each core processes B block-slots; for each it computes the
    [<=128, 8192] cosine block with fp8 DoubleRow matmuls (K=256 in one
    instruction), exp row-sums via ACT accum_out, and the same-range terms
    using the first-order expansion
        sum_j ln(negsum + exp(Cs_j)) ~= W_s*ln(negsum) + ss/negsum
    (exact to ~1e-7 since exp(C)/negsum <= 3e-4), so no per-element ln
    pass is needed. csr = sum_j Cs_j comes from a tiny matmul against
    host-precomputed same-range column sums.
  * Host: inter = sum(terms)/bs^2; l1/l2 finalized from S in float64
    (O(bs*L) work); loss = 0.5*inter + 0.5*(l1+l2).

Both launches use only Exp/Ln/Identity/Square activations; a compile-time
activation-table override makes bass pick the combined
natural_log_exp_and_others set so each launch issues exactly one
ACT_TABLE_LOAD (the greedy default alternates sets, costing ~1.4us per
switch).
"""

import contextlib
import math
import os

import ml_dtypes
import numpy as np

os.environ.setdefault("NEURON_RT_VIRTUAL_CORE_SIZE", "1")

import concourse.bass as bass
import concourse.mybir as mybir
from concourse import bacc
import concourse.tile as tile
from concourse.bass_utils import run_bass_kernel_spmd

BS = 8192
D_IN = 1024
D_EMB = 256
L = 10
NC = 8
P = 128
RPC = BS // NC          # rows per core in launch A
RT = RPC // P           # row tiles per core (8)
KT = D_IN // P          # k tiles (8)
KM = D_EMB // P         # emb-dim partition chunks (2)
GA = 2                  # launch A row groups (512 rows each)
RG = RPC // GA

F32 = mybir.dt.float32
BF16 = mybir.dt.bfloat16
BF16_NP = ml_dtypes.bfloat16
F8 = mybir.dt.float8e4
F8_NP = ml_dtypes.float8_e4m3
F8_SCALE = 16.0
AX = mybir.AxisListType.X
AF = mybir.ActivationFunctionType
DR = mybir.MatmulPerfMode.DoubleRow

# Results of the last kernel() call (for test.py introspection/timing).
LAST = {}


@contextlib.contextmanager
def _combined_act_tables():
    """Make the act-table pass resolve Exp and Ln to the combined
    natural_log_exp_and_others set (index preserved), so kernels using
    both emit a single ACT_TABLE_LOAD instead of alternating sets."""
    orig = bacc.get_activation_tables

    def patched(arch):
        t = orig(arch)
        combined = t["natural_log_exp_and_others"]
        return {
            k: (v if k == "natural_log_exp_and_others" else v - combined)
            for k, v in t.items()
        }

    bacc.get_activation_tables = patched
    try:
        yield
    finally:
        bacc.get_activation_tables = orig


# --------------------------------------------------------------------------
# Launch A: per-core transform  embT[1024,1024]bf16 -> enT[256,1024] bf16,
# S'[1024,10] f32 (unnormalized), rni[1024] f32
# --------------------------------------------------------------------------
def build_launch_a():
    nc = bacc.Bacc("TRN2", target_bir_lowering=False, debug=False, num_devices=NC)
    embt_d = nc.dram_tensor("embt", [P, GA * KT * RG], BF16, kind="ExternalInput")
    wt_d = nc.dram_tensor("wt", [P, KT * D_EMB], BF16, kind="ExternalInput")
    bias_d = nc.dram_tensor("bias", [P, KM], F32, kind="ExternalInput")
    lnt_d = nc.dram_tensor("lnt", [P, KM * L], F32, kind="ExternalInput")
    ent_d = nc.dram_tensor("ent_out", [P, KM * RPC], BF16, kind="ExternalOutput")
    s_d = nc.dram_tensor("s_out", [P, RT * L], F32, kind="ExternalOutput")
    rni_d = nc.dram_tensor("rni_out", [1, RPC], F32, kind="ExternalOutput")

    NCH = RPC // 512        # 512-wide column chunks of eT (2)
    with tile.TileContext(nc) as tc:
        with (
            tc.tile_pool(name="const", bufs=1) as cpool,
            tc.tile_pool(name="big", bufs=1) as big_pool,
            tc.tile_pool(name="sq", bufs=4) as sq_pool,
            tc.tile_pool(name="sml", bufs=2) as sml_pool,
            tc.tile_pool(name="ps", bufs=1, space="PSUM") as ps_pool,
        ):
            ones_c = cpool.tile([P, 1], BF16)
            nc.vector.memset(ones_c[:], 1.0)
            ones_r = cpool.tile([1, P], F32)
            nc.vector.memset(ones_r[:], 1.0)

            wt_sb = cpool.tile([P, KT * D_EMB], BF16)
            nc.gpsimd.dma_start(wt_sb[:], wt_d.ap())
            b_sb = cpool.tile([P, KM], F32)
            nc.gpsimd.dma_start(b_sb[:], bias_d.ap())
            lnt_sb = cpool.tile([P, KM * L], F32)
            nc.gpsimd.dma_start(lnt_sb[:], lnt_d.ap())

            embt_sb = big_pool.tile([P, GA, KT, RG], BF16)
            for g in range(GA):
                for h in range(2):  # kc-halves so matmuls start early
                    nc.gpsimd.dma_start(
                        embt_sb[:, g, h * KT // 2:(h + 1) * KT // 2, :],
                        embt_d.ap()[:, (2 * g + h) * KT * RG // 2:
                                    (2 * g + h + 1) * KT * RG // 2],
                    )

            # --- eT = W @ embT + b; norms^2 via ones-matmul over squares ---
            eT = [big_pool.tile([P, RPC], F32, name=f"eT{m}") for m in range(KM)]
            psn = [ps_pool.tile([1, 512], F32, name=f"psn{g}", tag=f"psn{g}")
                   for g in range(GA)]
            for g in range(GA):
                for m in range(KM):
                    pe = ps_pool.tile([P, 512], F32, tag="peA", bufs=2)
                    for kc in range(KT):
                        nc.tensor.matmul(
                            pe[:],
                            wt_sb[:, kc * D_EMB + m * P: kc * D_EMB + (m + 1) * P],
                            embt_sb[:, g, kc, :],
                            start=(kc == 0),
                            stop=(kc == KT - 1),
                        )
                    nc.scalar.activation(
                        eT[m][:, g * 512:(g + 1) * 512], pe[:],
                        AF.Identity, bias=b_sb[:, m:m + 1],
                    )
                    esq = sq_pool.tile([P, 512], BF16)
                    nc.vector.tensor_mul(
                        esq[:],
                        eT[m][:, g * 512:(g + 1) * 512],
                        eT[m][:, g * 512:(g + 1) * 512],
                    )
                    nc.tensor.matmul(
                        psn[g][:],
                        ones_c[:],
                        esq[:],
                        start=(m == 0),
                        stop=(m == KM - 1),
                    )

            # --- rni = exp(-0.5 * ln(|e|^2)) on ACT (all lanes idle; [1,*]) ---
            lnsq = sml_pool.tile([1, RPC], F32)
            for g in range(GA):
                nc.scalar.activation(
                    lnsq[:, g * 512:(g + 1) * 512], psn[g][:], AF.Ln,
                )
            rni = sml_pool.tile([1, RPC], F32)
            nc.scalar.activation(rni[:], lnsq[:], AF.Exp, scale=-0.5)
            nc.gpsimd.dma_start(rni_d.ap(), rni[:])

            # --- S' = eT.T @ lnT (unnormalized; fp32, tiny free dim) ---
            s_sb = sml_pool.tile([P, RT * L], F32)
            for r in range(RT):
                pss = ps_pool.tile([P, L], F32, tag="pss", bufs=2)
                for m in range(KM):
                    nc.tensor.matmul(
                        pss[:],
                        eT[m][:, r * P:(r + 1) * P],
                        lnt_sb[:, m * L:(m + 1) * L],
                        start=(m == 0),
                        stop=(m == KM - 1),
                    )
                nc.vector.tensor_copy(s_sb[:, r * L:(r + 1) * L], pss[:])
            nc.gpsimd.dma_start(s_d.ap(), s_sb[:])

            # --- enT = eT * rni (broadcast via K=1 matmul), bf16 out;
            #     output DMA chunked per (g, m) to overlap the tail ---
            enT_sb = big_pool.tile([P, KM * RPC], BF16)
            for g in range(GA):
                psb = ps_pool.tile([P, 512], F32, tag="psb", bufs=2)
                nc.tensor.matmul(
                    psb[:], ones_r[:], rni[:, g * 512:(g + 1) * 512],
                    start=True, stop=True,
                )
                for m in range(KM):
                    lo = m * RPC + g * 512
                    nc.vector.tensor_mul(
                        enT_sb[:, lo:lo + 512],
                        eT[m][:, g * 512:(g + 1) * 512],
                        psb[:],
                    )
                    nc.gpsimd.dma_start(
                        ent_d.ap()[:, lo:lo + 512], enT_sb[:, lo:lo + 512]
                    )

    with _combined_act_tables():
        nc.compile()
    return nc


# --------------------------------------------------------------------------
# Launch B: per-core B block-slots of the inter-sample loss
# --------------------------------------------------------------------------
def build_launch_b(B, W_s):
    WH = (W_s + 511) // 512  # samerange 512-col chunks (last may be partial)
    nc = bacc.Bacc("TRN2", target_bir_lowering=False, debug=False, num_devices=NC)
    ent_d = nc.dram_tensor("ent", [P, KM * BS], F8, kind="ExternalInput")
    lhst_d = nc.dram_tensor("lhst", [P, KM * B * P], F8, kind="ExternalInput")
    rs_d = nc.dram_tensor("rsame", [P, KM * B * W_s], F8, kind="ExternalInput")
    wsum_d = nc.dram_tensor("wsum", [P, KM * B], F8, kind="ExternalInput")
    meta_d = nc.dram_tensor("meta", [P, 3 * B], F32, kind="ExternalInput")
    terms_d = nc.dram_tensor("terms", [P, B], F32, kind="ExternalOutput")

    CW = 2048                  # psum/ACT chunk width (4 banks)
    NG = BS // CW              # main chunk groups (4)
    INV_FS2 = 1.0 / (F8_SCALE * F8_SCALE)
    with tile.TileContext(nc) as tc:
        with (
            tc.tile_pool(name="inp", bufs=1) as inp_pool,
            tc.tile_pool(name="escr", bufs=2) as escr_pool,
            tc.tile_pool(name="sml", bufs=2) as sml_pool,
            tc.tile_pool(name="psm", bufs=2, space="PSUM") as psm_pool,
        ):
            ent_sb = inp_pool.tile([P, NG, KM, CW], F8)
            lhst_sb = inp_pool.tile([P, KM, B, P], F8)
            # rsame is b-outer [P, (b, m, w)] so per-b DMA chunks are contiguous
            rs_sb = inp_pool.tile([P, B, KM, W_s], F8)
            wsum_sb = inp_pool.tile([P, KM, B], F8)
            meta_sb = inp_pool.tile([P, 3 * B], F32)
            nc.gpsimd.dma_start(meta_sb[:], meta_d.ap())
            nc.gpsimd.dma_start(wsum_sb[:, :, :], wsum_d.ap())
            nc.gpsimd.dma_start(lhst_sb[:, :, :, :], lhst_d.ap())
            # rsame chunked by b-thirds so block 0 can start early
            third = max(1, B // 3)
            bounds = [0, third, 2 * third, B]
            for i in range(3):
                lo, hi = bounds[i], bounds[i + 1]
                if lo < hi:
                    nc.gpsimd.dma_start(
                        rs_sb[:, lo:hi, :, :],
                        rs_d.ap()[:, lo * KM * W_s: hi * KM * W_s],
                    )
            # ent grouped by 2048-column blocks: [(g, m) -> CW cols]; chunked
            # DMAs so main-row group g can start once its chunk lands
            for g in range(NG):
                nc.gpsimd.dma_start(
                    ent_sb[:, g, :, :],
                    ent_d.ap()[:, g * KM * CW:(g + 1) * KM * CW],
                )
            coef_sb = meta_sb[:, 0:B]
            pad_sb = meta_sb[:, B:2 * B]
            mask_sb = meta_sb[:, 2 * B:3 * B]
            terms_sb = inp_pool.tile([P, B], F32)

            # per-block accumulators, assembled in one vectorized end pass
            ss_all = inp_pool.tile([P, B], F32)
            csr_all = inp_pool.tile([P, B], F32)
            rsp_all = inp_pool.tile([P, B, NG], F32)
            # csr matmul output goes in an otherwise-unused PSUM bank of ps_s
            pw_col = ((W_s + 511) // 512) * 512

            for b in range(B):
                lhs = lhst_sb[:, :, b, :]        # [128, 2, 128] fp8

                ps_s = psm_pool.tile([P, CW], F32, tag="psbig", bufs=2)
                ps_w = ps_s[:, pw_col:pw_col + 1]
                for m in range(KM):
                    nc.tensor.matmul(
                        ps_w,
                        lhst_sb[:, m, b, :],
                        wsum_sb[:, m, b:b + 1],
                        start=(m == 0), stop=(m == KM - 1),
                    )
                # same-label column range: C block + exp (accum=ss)
                for h in range(WH):
                    w0 = h * 512
                    w1 = min(W_s, w0 + 512)
                    nc.tensor.matmul(
                        ps_s[:, w0:w1],
                        lhs,
                        rs_sb[:, b, :, w0:w1],
                        start=True, stop=True, perf_mode=DR,
                    )
                es = escr_pool.tile([P, W_s], BF16, tag="es", bufs=2)
                nc.scalar.activation(
                    es[:], ps_s[:, :W_s], AF.Exp, accum_out=ss_all[:, b:b + 1],
                    scale=INV_FS2,
                )
                nc.vector.tensor_scalar_mul(csr_all[:, b:b + 1], ps_w, INV_FS2)

                # full row: C chunks + exp row-sums
                for g in range(NG):
                    ps_c = psm_pool.tile([P, CW], F32, tag="psbig", bufs=2)
                    for nn in range(CW // 512):
                        nc.tensor.matmul(
                            ps_c[:, nn * 512:(nn + 1) * 512],
                            lhs,
                            ent_sb[:, g, :, nn * 512:(nn + 1) * 512],
                            start=True, stop=True, perf_mode=DR,
                        )
                    e_scr = escr_pool.tile([P, CW], BF16)
                    nc.scalar.activation(
                        e_scr[:], ps_c[:], AF.Exp,
                        accum_out=rsp_all[:, b, g:g + 1],
                        scale=INV_FS2,
                    )

            # ---- vectorized assembly over all B blocks ----
            rs_all = sml_pool.tile([P, B], F32)
            nc.vector.tensor_add(rs_all[:], rsp_all[:, :, 0], rsp_all[:, :, 1])
            for g in range(2, NG):
                nc.vector.tensor_add(rs_all[:], rs_all[:], rsp_all[:, :, g])
            negsum = sml_pool.tile([P, B], F32)
            nc.vector.tensor_sub(negsum[:], rs_all[:], ss_all[:])
            nc.vector.tensor_add(negsum[:], negsum[:], pad_sb)

            # ln(negsum+1), ln(negsum+e), ln(negsum) in one ACT call
            ladd = sml_pool.tile([P, 3 * B], F32)
            nc.vector.tensor_scalar_add(ladd[:, 0:B], negsum[:], 1.0)
            nc.vector.tensor_scalar_add(ladd[:, B:2 * B], negsum[:], float(np.e))
            nc.vector.tensor_copy(ladd[:, 2 * B:3 * B], negsum[:])
            lout = sml_pool.tile([P, 3 * B], F32)
            nc.scalar.activation(lout[:], ladd[:], AF.Ln)
            lt = lout[:, 0:B]
            le = lout[:, B:2 * B]
            lnn = lout[:, 2 * B:3 * B]

            # first-order: sum_j ln(negsum+exp(Cs_j)) = W_s*lnn + ss/negsum
            rec = sml_pool.tile([P, B], F32)
            nc.vector.reciprocal(rec[:], negsum[:])
            lnsum = sml_pool.tile([P, B], F32)
            nc.vector.tensor_mul(lnsum[:], ss_all[:], rec[:])
            wlnn = sml_pool.tile([P, B], F32)
            nc.vector.tensor_scalar_mul(wlnn[:], lnn, float(W_s))
            nc.vector.tensor_add(lnsum[:], lnsum[:], wlnn[:])

            # term = coef*lt + (lnsum - csr) - le + 1, masked
            t1 = sml_pool.tile([P, B], F32)
            nc.vector.tensor_mul(t1[:], coef_sb, lt)
            nc.vector.tensor_add(t1[:], t1[:], lnsum[:])
            nc.vector.tensor_sub(t1[:], t1[:], csr_all[:])
            nc.vector.tensor_sub(t1[:], t1[:], le)
            nc.vector.tensor_scalar_add(t1[:], t1[:], 1.0)
            nc.vector.tensor_mul(terms_sb[:], t1[:], mask_sb)

            nc.gpsimd.dma_start(terms_d.ap(), terms_sb[:])

    with _combined_act_tables():
        nc.compile()
    return nc


# --------------------------------------------------------------------------
# Host orchestration
# --------------------------------------------------------------------------
def _plan_blocks(labels_s):
    counts = np.bincount(labels_s.astype(np.int64), minlength=L)
    starts = np.concatenate([[0], np.cumsum(counts)[:-1]])
    blocks = []
    for lab in range(L):
        s, c = int(starts[lab]), int(counts[lab])
        for off in range(0, c, P):
            blocks.append((s + off, min(P, c - off), lab))
    B = math.ceil(len(blocks) / NC)
    W_s = max(512, math.ceil((int(counts.max()) if len(blocks) else 1) / 128) * 128)
    return blocks, counts, starts, B, W_s


def _pm(a):
    """[G, P, N] -> partition-major [P, G*N]."""
    g, p, n = a.shape
    return np.ascontiguousarray(a.transpose(1, 0, 2).reshape(p, g * n))


def _prep_launch_a_inputs(emb_s, W, b, label_emb):
    wt = _pm(np.ascontiguousarray(W.T).reshape(KT, P, D_EMB)).astype(BF16_NP)
    bias = np.ascontiguousarray(b.reshape(KM, P).T).astype(np.float32)
    ln = label_emb / np.maximum(
        np.sqrt((label_emb.astype(np.float64) ** 2).sum(-1, keepdims=True)), 1e-8
    )
    lnt = _pm(np.ascontiguousarray(ln.T).reshape(KM, P, L)).astype(np.float32)
    in_maps = []
    for c in range(NC):
        sh = emb_s[c * RPC:(c + 1) * RPC].astype(BF16_NP)  # [1024, 1024] bf16
        # embT layout [P, (g, kc, r)]: embt[p, g, kc, r] = sh[g*RG + r, kc*128+p]
        et = np.ascontiguousarray(
            sh.reshape(GA, RG, KT, P).transpose(3, 0, 2, 1).reshape(P, GA * KT * RG)
        )
        in_maps.append({"embt": et, "wt": wt, "bias": bias, "lnt": lnt})
    return in_maps


def _prep_launch_b_inputs(enT_full, blocks, counts, starts, B, W_s):
    CW = 2048
    NG = BS // CW
    entf = enT_full.astype(np.float32)
    ent8 = (entf * F8_SCALE).astype(F8_NP)
    ent3 = ent8.reshape(KM, P, BS)
    # [P, (g, m, cw)] grouping to match the chunked DMAs
    ent = np.ascontiguousarray(
        ent8.reshape(KM, P, NG, CW).transpose(1, 2, 0, 3).reshape(P, NG * KM * CW)
    )
    in_maps = []
    for c in range(NC):
        blks = blocks[c * B:(c + 1) * B]
        lhst = np.zeros((KM, P, B * P), F8_NP)
        rsame = np.zeros((KM, P, B, W_s), F8_NP)
        wsum = np.zeros((KM, P, B), np.float32)
        meta = np.zeros((P, 3 * B), np.float32)
        for i, (rs, w, lab) in enumerate(blks):
            lhst[:, :, i * P:i * P + w] = ent3[:, :, rs:rs + w]
            s, cnt = int(starts[lab]), int(counts[lab])
            rsame[:, :, i, :cnt] = ent3[:, :, s:s + cnt]
            # same-range column sums (x F8_SCALE) for the csr matmul
            wsum[:, :, i] = (
                entf.reshape(KM, P, BS)[:, :, s:s + cnt].sum(axis=2) * F8_SCALE
            )
            meta[:w, i] = BS - W_s            # coef
            meta[:, B + i] = W_s - cnt        # pad
            meta[:w, 2 * B + i] = 1.0         # mask
        rsame_pm = np.ascontiguousarray(
            rsame.transpose(1, 2, 0, 3).reshape(P, B * KM * W_s)
        )
        in_maps.append({
            "ent": ent, "lhst": _pm(lhst), "rsame": rsame_pm,
            "wsum": _pm(wsum.astype(F8_NP)), "meta": meta,
        })
    return in_maps


def _finalize_l1_l2(S_sorted, labels_s):
    S = S_sorted.astype(np.float64)
    idx = np.arange(BS)
    lab = labels_s.astype(np.int64)
    Pv = S[idx, lab]
    E2 = np.exp(S)
    eP = np.exp(Pv)
    neg1 = E2.sum(axis=1) - eP
    col_tot = E2.sum(axis=0)
    own_col = np.bincount(lab, weights=eP, minlength=L)
    neg2 = (col_tot - own_col)[lab]
    l1 = np.mean(-Pv + np.log(neg1 + eP))
    l2 = np.mean(-Pv + np.log(neg2 + eP))
    return l1, l2


def kernel(embedding, labels, W, b, label_emb):
    embedding = np.asarray(embedding, np.float32)
    labels_np = np.asarray(labels)
    W = np.asarray(W, np.float32)
    b = np.asarray(b, np.float32)
    label_emb = np.asarray(label_emb, np.float32)

    perm = np.argsort(labels_np, kind="stable")
    labels_s = labels_np[perm]
    emb_s = embedding[perm]
    blocks, counts, starts, B, W_s = _plan_blocks(labels_s)

    # ---- launch A ----
    nc_a = build_launch_a()
    in_maps_a = _prep_launch_a_inputs(emb_s, W, b, label_emb)
    res_a = run_bass_kernel_spmd(nc_a, in_maps_a, core_ids=list(range(NC)))
    LAST["a"] = res_a

    enT_full = np.empty((D_EMB, BS), BF16_NP)
    S_sorted = np.empty((BS, L), np.float64)
    for c in range(NC):
        out = res_a.results[c]
        ent_c = np.asarray(out["ent_out"])  # [P, KM*RPC]
        for m in range(KM):
            enT_full[m * P:(m + 1) * P, c * RPC:(c + 1) * RPC] = \
                ent_c[:, m * RPC:(m + 1) * RPC]
        s_c = np.asarray(out["s_out"]).reshape(P, RT, L)
        rni_c = np.asarray(out["rni_out"]).reshape(RPC)
        S_sorted[c * RPC:(c + 1) * RPC] = (
            s_c.transpose(1, 0, 2).reshape(RPC, L).astype(np.float64)
            * rni_c[:, None].astype(np.float64)
        )

    # ---- launch B ----
    nc_b = build_launch_b(B, W_s)
    in_maps_b = _prep_launch_b_inputs(enT_full, blocks, counts, starts, B, W_s)
    res_b = run_bass_kernel_spmd(nc_b, in_maps_b, core_ids=list(range(NC)))
    LAST["b"] = res_b

    total = 0.0
    for c in range(NC):
        total += np.asarray(res_b.results[c]["terms"], np.float64).sum()
    inter = total / (BS * BS)

    l1, l2 = _finalize_l1_l2(S_sorted, labels_s)
    return np.float32(0.5 * inter + 0.5 * (l1 + l2))
